# revision 1
# baseline (speedup 1.0000x reference)
# Trainium2 Bass kernel for nn_Net_4861902979707
#
# Computation (per sample, B = 4194304):
#   X [B, 3, 3] -> 3 pairwise Euclidean distances d = [d01, d02, d12]
#   h1 = elu(d @ W1.T + b1); h2 = elu(h1 @ W2.T + b2); y = h2 @ W3.T + b3
#
# v2 design (faster than the previous diag-matmul baseline):
#  - fp16 end-to-end on chip (X host-cast to fp16: halves HBM read traffic,
#    4x less rounding error than bf16). PSUM accumulation stays fp32.
#  - interleaved [P, T, k] layouts so every DVE tensor_tensor op qualifies
#    for the 2x packed mode (2-byte dtype, innermost stride 1) and every
#    tensor_scalar for 4x.
#  - ELU via the exact identity elu(v)+1 = max(v+1, min(exp(v), 1)):
#      e   = exp(z+b)          (ACT, reads PSUM once, bias fused)
#      ec  = min(e, 1)         (DVE tensor_scalar 4x, or Pool)
#      h   = max(z+(b+1), ec)  (DVE scalar_tensor_tensor from PSUM)
#        or s = z+(b+1) on ACT then max(s, ec) on Pool (gpsimd has no
#        PSUM port, so Pool only ever touches SBUF operands).
#    The +1 shift is absorbed into the next layer's bias (b' = b - W @ 1).
#  - two-phase superblocks: the ACT engine's Sqrt and Exp live in different
#    hardware activation-table sets (a switch costs ~1.3us), so tiles are
#    processed in superblocks of SB tiles: distances (Sqrt) for all SB tiles,
#    then the MLP (Exp) for the same tiles. 2 table switches per superblock
#    instead of 2 per tile.
#  - PE does every linear reduction as diagonal-matmul PSUM accumulations:
#    coord sums (identity lhsT) and the three MLP layers (W*I lhsT).
import os as _os
import numpy as np

B = 4194304
N_CORES = 8
B_CORE = B // N_CORES          # 524288
P = 128                        # partitions
T = 512                        # samples per partition per tile (PE max move)
TILE = P * T                   # 65536
N_TILES = B_CORE // TILE       # 8

SB = int(_os.environ.get("SB", "1"))        # tiles per ACT-table superblock
SQA = int(_os.environ.get("SQA", "0"))      # square elems on ACT (0..9)
N_STT = int(_os.environ.get("N_STT", "4"))  # ELU units on the DVE-stt path
EC_ENGINE = _os.environ.get("EC_ENGINE", "dve")  # pool | dve
L3_MODE = _os.environ.get("L3_MODE", "pe_act")  # pe_act | pe_dve | dve
MERGED = _os.environ.get("MERGED", "0") == "1"  # per-layer merged ELU ops
STAGE = _os.environ.get("STAGE", "full")    # full | dma | dist
BUFS_X = int(_os.environ.get("BUFS_X", "3"))
BUFS_D = int(_os.environ.get("BUFS_D", "3"))
BUFS_DIST = int(_os.environ.get("BUFS_DIST", "9"))
BUFS_M = int(_os.environ.get("BUFS_M", "6"))

_CACHE = {}


def _split_sync_waits(nc, mybir, limit=1):
    """This walrus build rejects instructions carrying more than ~1 sem wait
    ("Too many sync wait commands"). Hoist excess waits onto NoOp carrier
    instructions (same engine, immediately before) — engine program order
    preserves the blocking semantics."""
    n_split = 0
    for f in nc.m.functions:
        for b in f.blocks:
            lst = b.instructions
            out = []
            changed = False
            for inst in lst:
                si = inst.sync_info
                if si is not None and len(si.on_wait) > limit:
                    waits = list(si.on_wait)
                    extra, keep = waits[:-limit], waits[-limit:]
                    for wi, w in enumerate(extra):
                        nop = mybir.InstNoOp(
                            name=f"wsplit-{inst.name}-{wi}")
                        nop.engine = inst.engine
                        nop.sync_info = mybir.SyncInfo(
                            on_wait=[w], on_update=[])
                        out.append(nop)
                        n_split += 1
                    inst.sync_info = type(si)(
                        on_wait=keep, on_update=list(si.on_update))
                    changed = True
                out.append(inst)
            if changed:
                b.instructions = out
    return n_split


# WD diag-matrix indices (each a [128,128] fp16 lhsT)
def _iWD_I():
    return 0
def _iWD_W1(k, j):
    return 1 + 3 * k + j
def _iWD_W2(m, j):
    return 7 + 2 * m + j
def _iWD_W3(j):
    return 11 + j
def _iWD_B1(k):   # diag(b1[k]+1)
    return 13 + k
def _iWD_B2(m):   # diag(b2'[m]+1)
    return 15 + m
N_WD = 17

# WB scalar indices ([P,1] fp32 broadcast):
#   b1[k]=k, b2'[m]=2+m, b3'=4, b1[k]+1=5+k, b2'[m]+1=7+m, w30=9, w31=10
def _ib1(k):
    return k
def _ib2(m):
    return 2 + m
_IB3 = 4
def _ib1p(k):
    return 5 + k
def _ib2p(m):
    return 7 + m
_IW30 = 9
_IW31 = 10
_IM1 = 11   # constant -1.0
N_WB = 12


def _build(reps=1, bench_small=False):
    import concourse.bass as bass
    import concourse.tile as tile
    import concourse.mybir as mybir
    from concourse.dve_ops import AFFINE_THEN_ADD

    f32 = mybir.dt.float32
    f16 = mybir.dt.float16
    Alu = mybir.AluOpType
    Act = mybir.ActivationFunctionType

    nc = bass.Bass()
    BC = TILE if bench_small else B_CORE
    X = nc.dram_tensor("X", [BC, 9], f16, kind="ExternalInput")
    WB = nc.dram_tensor("WB", [N_WB], f32, kind="ExternalInput")
    WD = nc.dram_tensor("WD", [N_WD, P, P], f16, kind="ExternalInput")
    Y = nc.dram_tensor("Y", [BC, 1], f16, kind="ExternalOutput")

    # which ELU units take the DVE-stt path: order [L1u0, L1u1, L2u0, L2u1]
    unit_stt = [i < N_STT for i in range(4)]

    with tile.TileContext(nc) as tc:
        with (
            tc.tile_pool(name="singles", bufs=1) as singles,
            tc.tile_pool(name="xin", bufs=BUFS_X) as xin,
            tc.tile_pool(name="dif", bufs=BUFS_D) as dif,
            tc.tile_pool(name="distp", bufs=BUFS_DIST) as distp,
            tc.tile_pool(name="mlp", bufs=BUFS_M) as mlp,
            tc.tile_pool(name="yout", bufs=6) as yout,
            tc.tile_pool(name="psum", bufs=1, space="PSUM") as psum,
        ):
            # broadcast bias scalars to all partitions; load diag matrices
            wb = singles.tile([P, N_WB], f32)
            nc.gpsimd.dma_start(
                out=wb[:],
                in_=bass.AP(tensor=WB[:].tensor, offset=0,
                            ap=[[0, P], [1, N_WB]]))
            wd = singles.tile([P, N_WD, P], f16)
            nc.sync.dma_start(
                out=wd[:],
                in_=bass.AP(tensor=WD[:].tensor, offset=0,
                            ap=[[P, P], [P * P, N_WD], [1, P]]))

            def ws(i):  # [P,1] scalar AP
                return wb[:, i:i + 1]

            def diag(i):  # [128,128] lhsT AP
                return wd[:, i, :]

            _loop = tc.For_i(0, reps) if reps != 1 else None
            if _loop is not None:
                _loop.__enter__()

            # constant ones plane for the bias matmuls
            ones = singles.tile([P, T], f16)
            nc.vector.memset(ones[:], 1.0)

            def elu_unit(z, tag):
                """h = elu(v) + 1 = max(v+1, min(exp(v), 1)), where the PSUM
                tile z already holds v+1 (the +b+1 bias was accumulated on PE
                via a diag(b+1) @ ones matmul). Two ops total:
                  e = exp(z - 1)              (ACT, one PSUM read)
                  h = (e min 1) max z         (DVE stt, fused clamp+combine)
                fp16 overflow of exp is benign: inf min 1 = 1."""
                et = mlp.tile([P, T], f16, tag=f"e_{tag}")
                nc.scalar.activation(
                    et, z[:], Act.Exp, bias=ws(_IM1), scale=1.0)
                h = mlp.tile([P, T], f16, tag=f"h_{tag}")
                nc.vector.scalar_tensor_tensor(
                    out=h, in0=et, scalar=1.0, in1=z[:],
                    op0=Alu.min, op1=Alu.max)
                return h

            def mlp_layer(inp, widx_fn, bidx_fn, nin, ztag):
                """One MLP layer, both units at once.
                z'' = W @ x + (b+1) accumulated on PE (bias via a ones-plane
                matmul with a diag(b+1) lhsT), then elu(v)+1 =
                max(v+1, min(exp(v), 1)) with ONE exp / ONE clamp / ONE
                combine over [P, 2, T] (v+1 = z''):
                  e  = exp(z'' - 1)            (ACT, single PSUM read)
                  ec = min(e, 1)               (DVE tensor_scalar 4x)
                  h  = max(z'' + 0, ec)        (DVE stt from PSUM)
                fp16 overflow of exp is benign: inf min 1 = 1."""
                z = psum.tile([P, 2, T], f32, tag=ztag)
                for k in range(2):
                    for j in range(nin):
                        nc.tensor.matmul(
                            z[:, k, :], diag(widx_fn(k, j)), inp(j),
                            start=(j == 0), stop=False)
                    nc.tensor.matmul(
                        z[:, k, :], diag(bidx_fn(k)), ones[:],
                        start=False, stop=True)
                et = mlp.tile([P, 2, T], f16, tag=f"e_{ztag}")
                nc.scalar.activation(
                    et[:], z[:], Act.Exp, bias=ws(_IM1), scale=1.0)
                ec = mlp.tile([P, 2, T], f16, tag=f"ec_{ztag}")
                eng = nc.gpsimd if EC_ENGINE == "pool" else nc.vector
                eng.tensor_scalar(
                    out=ec[:], in0=et[:], scalar1=1.0, scalar2=0.0,
                    op0=Alu.min, op1=Alu.bypass)
                h = mlp.tile([P, 2, T], f16, tag=f"h_{ztag}")
                nc.vector.scalar_tensor_tensor(
                    out=h[:], in0=z[:], scalar=0.0, in1=ec[:],
                    op0=Alu.add, op1=Alu.max)
                return h

            all_sb = [list(range(s, min(s + SB, N_TILES)))
                      for s in range(0, N_TILES, SB)]
            sb_dist = {}

            # ---- phase A: input -> distances (ACT uses Sqrt table) ----
            def emit_A(tiles):
                for ti in tiles:
                    src = 0 if bench_small else ti
                    xr = X[src * TILE:(src + 1) * TILE, :].rearrange(
                        "(p s) d -> p s d", p=P)
                    yr = Y[src * TILE:(src + 1) * TILE, :].rearrange(
                        "(p s) d -> p (s d)", p=P)
                    xt = xin.tile([P, T, 9], f16)
                    nc.sync.dma_start(out=xt[:], in_=xr)

                    if STAGE == "dma":
                        yt = yout.tile([P, T], f16)
                        nc.scalar.activation(yt, xt[:, :, 0], Act.Copy)
                        nc.sync.dma_start(out=yr, in_=yt[:])
                        continue

                    # pairwise diffs, interleaved [P, T, 3] slices (DVE 2x)
                    diff = dif.tile([P, T, 9], f16)
                    nc.vector.tensor_sub(
                        diff[:, :, 0:3], xt[:, :, 0:3], xt[:, :, 3:6])
                    nc.vector.tensor_sub(
                        diff[:, :, 3:6], xt[:, :, 0:3], xt[:, :, 6:9])
                    nc.vector.tensor_sub(
                        diff[:, :, 6:9], diff[:, :, 3:6], diff[:, :, 0:3])

                    # squares in place: first 9-SQA elems on DVE, rest on ACT
                    ndve = 9 - SQA
                    if ndve > 0:
                        pl = diff[:, :, 0:ndve]
                        nc.vector.tensor_mul(pl, pl, pl)
                    if SQA > 0:
                        pl = diff[:, :, ndve:9]
                        nc.scalar.activation(pl, pl, Act.Square)

                    # coord sums on PE: q_p = sum_c I @ sq[:, :, 3p+c]
                    dts = []
                    for pp in range(3):
                        q = psum.tile([P, T], f32, tag=f"q{pp}")
                        for c in range(3):
                            nc.tensor.matmul(
                                q[:], diag(_iWD_I()), diff[:, :, 3 * pp + c],
                                start=(c == 0), stop=(c == 2))
                        d_ = distp.tile([P, T], f16, tag=f"dist{pp}")
                        nc.scalar.activation(d_, q[:], Act.Sqrt)
                        dts.append(d_)
                    sb_dist[ti] = (dts, yr)

            # ---- phase B: distances -> MLP (ACT uses Exp table) ----
            def emit_B(tiles):
                # staged across the superblock (all L1s, then all L2s, then
                # all L3s) so PE streams the next tile's matmuls while this
                # tile's ELU glue completes — no per-tile PE stall at each
                # layer boundary, and PE stays p-state-ramped.
                if STAGE == "dist":
                    for ti in tiles:
                        dts, yr = sb_dist[ti]
                        yt = yout.tile([P, T], f16)
                        nc.scalar.activation(yt, dts[0][:], Act.Copy)
                        nc.sync.dma_start(out=yr, in_=yt[:])
                    return

                sb_h2 = {}
                if MERGED:
                    for ti in tiles:
                        dts, yr = sb_dist[ti]
                        h1 = mlp_layer(
                            lambda j: dts[j][:], _iWD_W1, _iWD_B1, 3, "z1")
                        h2 = mlp_layer(
                            lambda j: h1[:, j, :], _iWD_W2, _iWD_B2, 2, "z2")
                        sb_h2[ti] = [h2[:, 0, :], h2[:, 1, :]]
                else:
                    sb_h1 = {}
                    for ti in tiles:
                        dts, _ = sb_dist[ti]
                        h1l = []
                        for k in range(2):
                            z = psum.tile([P, T], f32, tag=f"z1_{k}")
                            for j in range(3):
                                nc.tensor.matmul(
                                    z[:], diag(_iWD_W1(k, j)), dts[j][:],
                                    start=(j == 0), stop=False)
                            nc.tensor.matmul(
                                z[:], diag(_iWD_B1(k)), ones[:],
                                start=False, stop=True)
                            h1l.append(elu_unit(z, f"1{k}"))
                        sb_h1[ti] = h1l
                    for ti in tiles:
                        h1l = sb_h1[ti]
                        h2l = []
                        for m in range(2):
                            z = psum.tile([P, T], f32, tag=f"z2_{m}")
                            for j in range(2):
                                nc.tensor.matmul(
                                    z[:], diag(_iWD_W2(m, j)), h1l[j][:],
                                    start=(j == 0), stop=False)
                            nc.tensor.matmul(
                                z[:], diag(_iWD_B2(m)), ones[:],
                                start=False, stop=True)
                            h2l.append(elu_unit(z, f"2{m}"))
                        sb_h2[ti] = [h2l[0][:], h2l[1][:]]

                for ti in tiles:
                    _, yr = sb_dist[ti]
                    h2s = sb_h2[ti]
                    yt = yout.tile([P, T], f16)
                    if L3_MODE == "dve":
                        # y = (h2_1*w31 + b3') + (h2_0*w30), no PSUM round-trip
                        u = mlp.tile([P, T], f16, tag="u")
                        nc.vector.tensor_scalar(
                            out=u, in0=h2s[0], scalar1=ws(_IW30),
                            scalar2=0.0, op0=Alu.mult, op1=Alu.bypass)
                        nc.vector._custom_dve(
                            AFFINE_THEN_ADD, out=yt, in0=h2s[1], in1=u,
                            s0=ws(_IW31), s1=ws(_IB3))
                    else:
                        yz = psum.tile([P, T], f32, tag="yz")
                        for j in range(2):
                            nc.tensor.matmul(
                                yz[:], diag(_iWD_W3(j)), h2s[j],
                                start=(j == 0), stop=(j == 1))
                        if L3_MODE == "pe_act":
                            nc.scalar.activation(
                                yt, yz[:], Act.Identity,
                                bias=ws(_IB3), scale=1.0)
                        else:  # pe_dve
                            nc.vector.tensor_scalar(
                                out=yt, in0=yz[:], scalar1=ws(_IB3),
                                scalar2=0.0, op0=Alu.add, op1=Alu.bypass)
                    nc.sync.dma_start(out=yr, in_=yt[:])

            # software-pipelined drive: emit phase A one superblock ahead of
            # phase B, so the in-order DVE queue always holds independent
            # next-superblock diff/square work while this superblock's ELU
            # glue waits on its ACT/PE producers.
            for _i, _ts in enumerate(all_sb):
                emit_A(_ts)
                if STAGE != "dma" and _i >= 1:
                    emit_B(all_sb[_i - 1])
            if STAGE != "dma":
                emit_B(all_sb[-1])

            if _loop is not None:
                _loop.__exit__(None, None, None)

    _split_sync_waits(nc, mybir, limit=1)
    return nc


def _pack_weights(W1, b1, W2, b2, W3, b3):
    W1 = np.asarray(W1, np.float32); b1 = np.asarray(b1, np.float32)
    W2 = np.asarray(W2, np.float32); b2 = np.asarray(b2, np.float32)
    W3 = np.asarray(W3, np.float32); b3 = np.asarray(b3, np.float32)
    b2a = b2 - W2.sum(axis=1)            # absorb the elu(+1) shift
    b3a = b3 - W3.sum(axis=1)
    wb = np.empty(N_WB, np.float32)
    wb[0:2] = b1
    wb[2:4] = b2a
    wb[4] = b3a[0]
    wb[5:7] = b1 + 1.0
    wb[7:9] = b2a + 1.0
    wb[9] = W3[0, 0]
    wb[10] = W3[0, 1]
    wb[11] = -1.0

    eye = np.eye(P, dtype=np.float32)
    wd = np.empty((N_WD, P, P), np.float32)
    wd[_iWD_I()] = eye
    for k in range(2):
        for j in range(3):
            wd[_iWD_W1(k, j)] = eye * W1[k, j]
    for m in range(2):
        for j in range(2):
            wd[_iWD_W2(m, j)] = eye * W2[m, j]
    for j in range(2):
        wd[_iWD_W3(j)] = eye * W3[0, j]
    for k in range(2):
        wd[_iWD_B1(k)] = eye * (b1[k] + 1.0)
    for m in range(2):
        wd[_iWD_B2(m)] = eye * (b2a[m] + 1.0)
    return wb, wd.astype(np.float16)


LAST_RESULTS = None  # BassKernelResults of the most recent run (for test.py)


def kernel(X, W1, b1, W2, b2, W3, b3):
    from concourse.bass_utils import run_bass_kernel_spmd
    global LAST_RESULTS

    X = np.ascontiguousarray(
        np.asarray(X, np.float32).reshape(B, 9)).astype(np.float16)
    wb, wd = _pack_weights(W1, b1, W2, b2, W3, b3)

    key = ("v2", 1)
    if key not in _CACHE:
        _CACHE[key] = _build()
    nc = _CACHE[key]

    in_maps = [
        {"X": X[c * B_CORE:(c + 1) * B_CORE], "WB": wb, "WD": wd}
        for c in range(N_CORES)
    ]
    res = run_bass_kernel_spmd(nc, in_maps, core_ids=list(range(N_CORES)))
    LAST_RESULTS = res
    out = np.concatenate([res.results[c]["Y"] for c in range(N_CORES)], axis=0)
    return out.reshape(B, 1).astype(np.float32)

